# revision 1
# baseline (speedup 1.0000x reference)
"""Trainium2 Bass kernel for nn_DeferredRender (4-level bilinear grid_sample sum).

"Mega-entry" single-gather design
---------------------------------
For pixel (u, v), level L uses gx_L = u*W_L - 0.5, x0_L = floor(gx_L) (same
for y). Given the finest-level cell (x0_0, y0_0), each coarser level's x0_L is
confined to {xb_L, xb_L+1} with xb_L = floor((x0_0 - 2^(L-1)) / 2^L) — exact
even at float-rounding edges, because u*1024 = 2^k * (u*W_L) in binary fp. So
a 3x3 super-patch of level L anchored at (yb_L, xb_L) covers every possible
2x2 footprint of the pixel at that level.

The host builds one fp16 table indexed by (r0, k0) = (y0_0+1, x0_0+1):

  entry = [ L0 2x2 patch [dx,dy,c]  :  32 fp16 ]
          [ L1 3x3 patch [dx,dy,c]  :  72 fp16 ]
          [ L2 3x3 patch [dx,dy,c]  :  72 fp16 ]
          [ L3 3x3 patch [dx,dy,c]  :  72 fp16 ]   = 248 fp16 = 496 B

with zeros for out-of-bounds texels, which implements grid_sample's zero
padding for free (no masks or clamps anywhere).

Device kernel (per core, 256 of 2048 rows, H-sharded 8 ways): per [128 x K]
pixel block, compute the L0 cell + per-level fractions on ACT/DVE, fetch one
496B entry per pixel via SWDGE indirect DMA ([128,1] indices per instruction —
the HW-supported form), then weighted-sum: L0 with 4 corner weights, L1-3 with
3-wide zero-stencil weights placed at offset ox_L = x0_L - xb_L in {0,1}.
fp16 MAC, fp32 output, channel-major store.
"""

import numpy as np

C = 8
FULL_H = 2048
FULL_W = 2048
N_CORES = 8
ROWS = FULL_H // N_CORES  # 256
K = 128  # pixels per block column chunk

_CACHED = {}

L0 = 1024
ENT = 248           # fp16 elems per entry
GRID = L0 + 1       # 1025 values of r0/k0


def _build_mega_table(tex0, tex1, tex2, tex3):
    texs = [np.asarray(t, np.float32) for t in (tex0, tex1, tex2, tex3)]
    g = GRID
    x0 = np.arange(-1, L0)  # [-1 .. 1023]
    out = np.zeros((g, g, ENT), np.float16)

    def put(level_tex, base, dst, di, dj):
        H = level_tex.shape[1]
        W = level_tex.shape[2]
        yy = base + di
        xx = base + dj
        yv = (yy >= 0) & (yy < H)
        xv = (xx >= 0) & (xx < W)
        yc = np.clip(yy, 0, H - 1)
        xc = np.clip(xx, 0, W - 1)
        vals = level_tex[:, yc[:, None], xc[None, :]].transpose(1, 2, 0)
        vals = vals * (yv[:, None, None] & xv[None, :, None])
        dst[...] = vals.astype(np.float16)

    v = out[:, :, 0:32].reshape(g, g, 2, 2, C)
    for dx in range(2):
        for dy in range(2):
            put(texs[0], x0, v[:, :, dx, dy, :], dy, dx)
    off = 32
    for li in range(1, 4):
        half = 1 << (li - 1)
        b = (x0 - half) >> li
        v = out[:, :, off:off + 72].reshape(g, g, 3, 3, C)
        for dx in range(3):
            for dy in range(3):
                put(texs[li], b, v[:, :, dx, dy, :], dy, dx)
        off += 72
    return np.ascontiguousarray(out.reshape(g * g, ENT))


def _build_nc(rows, width, kk):
    import concourse.bacc as bacc
    import concourse.bass as bass
    import concourse.mybir as mybir
    import concourse.tile as tile

    f32 = mybir.dt.float32
    f16 = mybir.dt.float16
    i32 = mybir.dt.int32
    Copy = mybir.ActivationFunctionType.Copy
    MUL = mybir.AluOpType.mult
    ADD = mybir.AluOpType.add
    SUB = mybir.AluOpType.subtract

    nc = bacc.Bacc("TRN2", target_bir_lowering=False, debug=False,
                   num_devices=N_CORES)
    u_d = nc.dram_tensor("u", [rows, width], f32, kind="ExternalInput")
    v_d = nc.dram_tensor("v", [rows, width], f32, kind="ExternalInput")
    tbl_d = nc.dram_tensor("tbl", [GRID * GRID, ENT], f16, kind="ExternalInput")
    out_d = nc.dram_tensor("out", [C, rows, width], f32, kind="ExternalOutput")

    with tile.TileContext(nc) as tc:
        with tc.tile_pool(name="main", bufs=2) as pool:
            for r0 in range(0, rows, 128):
                for w0 in range(0, width, kk):
                    u_t = pool.tile([128, kk], f32, tag="u")
                    v_t = pool.tile([128, kk], f32, tag="v")
                    nc.sync.dma_start(u_t[:], u_d.ap()[r0:r0 + 128, w0:w0 + kk])
                    nc.sync.dma_start(v_t[:], v_d.ap()[r0:r0 + 128, w0:w0 + kk])

                    def cell(src, w, tagp):
                        """k = round(u*w) (HW cvt rounds); f = u*w + 0.5 - k."""
                        s = pool.tile([128, kk], f32, tag=f"s{tagp}")
                        nc.scalar.activation(s[:], src[:], Copy,
                                             bias=0.0, scale=float(w))
                        ki = pool.tile([128, kk], i32, tag=f"ki{tagp}")
                        nc.vector.tensor_copy(ki[:], s[:])
                        kf = pool.tile([128, kk], f32, tag=f"kf{tagp}")
                        nc.vector.tensor_copy(kf[:], ki[:])
                        fr = pool.tile([128, kk], f32, tag=f"fr{tagp}")
                        nc.vector.scalar_tensor_tensor(
                            out=fr[:], in0=s[:], scalar=0.5, in1=kf[:],
                            op0=ADD, op1=SUB)
                        return kf, fr

                    kx0, fx0 = cell(u_t, L0, "x0")
                    ky0, fy0 = cell(v_t, L0, "y0")

                    # idx = ky0*GRID + kx0  (kx0/ky0 are already the +1-shifted
                    # grid coords: kx0 = floor(gx)+1)
                    idx = pool.tile([128, kk], i32, tag="idx")
                    nc.vector.scalar_tensor_tensor(
                        out=idx[:], in0=ky0[:], scalar=float(GRID),
                        in1=kx0[:], op0=MUL, op1=ADD)

                    patch = pool.tile([128, kk * ENT], f16, tag="patch")
                    p3 = patch[:].rearrange("p (k e) -> p k e", e=ENT)
                    for k in range(kk):
                        nc.gpsimd.indirect_dma_start(
                            out=p3[:, k, :],
                            out_offset=None,
                            in_=tbl_d.ap(),
                            in_offset=bass.IndirectOffsetOnAxis(
                                ap=idx[:, k:k + 1], axis=0),
                        )

                    acc = pool.tile([128, kk * C], f16, tag="acc")
                    pv = p3

                    # ---- L0: 4-corner MAC ----
                    gx0 = pool.tile([128, kk], f32, tag="gx0")
                    gy0 = pool.tile([128, kk], f32, tag="gy0")
                    nc.scalar.activation(gx0[:], fx0[:], Copy, bias=1.0,
                                         scale=-1.0)
                    nc.scalar.activation(gy0[:], fy0[:], Copy, bias=1.0,
                                         scale=-1.0)
                    w4 = pool.tile([128, 4 * kk], f16, tag="w4")
                    w4v = w4[:].rearrange("p (j k) -> p j k", j=4)
                    nc.vector.tensor_mul(w4v[:, 0, :], gx0[:], gy0[:])
                    nc.vector.tensor_mul(w4v[:, 1, :], gx0[:], fy0[:])
                    nc.vector.tensor_mul(w4v[:, 2, :], fx0[:], gy0[:])
                    nc.vector.tensor_mul(w4v[:, 3, :], fx0[:], fy0[:])
                    w4b = (w4[:].rearrange("p (j k) -> p j k", j=4)
                           .transpose([0, 2, 1]).unsqueeze(3)
                           .broadcast_to([128, kk, 4, C]))
                    l0v = p3[:, :, 0:32].rearrange("p k (j c) -> p k j c", c=C)
                    nc.vector.tensor_mul(l0v, w4b, l0v)
                    nc.vector.tensor_add(pv[:, :, 0:16], pv[:, :, 0:16],
                                         pv[:, :, 16:32])
                    nc.vector.tensor_add(pv[:, :, 0:8], pv[:, :, 0:8],
                                         pv[:, :, 8:16])
                    nc.vector.tensor_copy(acc[:], pv[:, :, 0:8])

                    # ---- L1..L3: 3x3 stencil MAC ----
                    off = 32
                    for li in range(1, 4):
                        half = float(1 << (li - 1))
                        inv = 1.0 / float(1 << li)
                        w3 = {}
                        for coord, src, k0f in (("x", u_t, kx0),
                                                ("y", v_t, ky0)):
                            kLf, frL = cell(src, L0 >> li, coord)
                            # xb = floor((k0 - 1 - half) * inv); bias centers
                            # the dyadic frac grid so round-nearest == floor.
                            bias = -((1.0 + half) * inv) - (0.5 - 0.5 * inv)
                            t = pool.tile([128, kk], f32, tag=f"t{coord}")
                            nc.scalar.activation(t[:], k0f[:], Copy,
                                                 bias=bias, scale=inv)
                            xbi = pool.tile([128, kk], i32, tag=f"xbi{coord}")
                            nc.vector.tensor_copy(xbi[:], t[:])
                            xbf = pool.tile([128, kk], f32, tag=f"xbf{coord}")
                            nc.vector.tensor_copy(xbf[:], xbi[:])
                            # ox = (kL - 1) - xb  in {0, 1}
                            ox = pool.tile([128, kk], f32, tag=f"ox{coord}")
                            nc.vector.scalar_tensor_tensor(
                                out=ox[:], in0=kLf[:], scalar=-1.0,
                                in1=xbf[:], op0=ADD, op1=SUB)
                            # stencil: s0=(1-ox)(1-f), s2=ox*f, s1=1-s0-s2
                            a = pool.tile([128, kk], f32, tag=f"a{coord}")
                            nc.scalar.activation(a[:], frL[:], Copy,
                                                 bias=1.0, scale=-1.0)
                            b = pool.tile([128, kk], f32, tag=f"b{coord}")
                            nc.scalar.activation(b[:], ox[:], Copy,
                                                 bias=1.0, scale=-1.0)
                            s0 = pool.tile([128, kk], f32, tag=f"s0{coord}")
                            nc.vector.tensor_mul(s0[:], b[:], a[:])
                            s2 = pool.tile([128, kk], f32, tag=f"s2{coord}")
                            nc.vector.tensor_mul(s2[:], ox[:], frL[:])
                            sm = pool.tile([128, kk], f32, tag=f"sm{coord}")
                            nc.scalar.activation(sm[:], s0[:], Copy,
                                                 bias=1.0, scale=-1.0)
                            s1 = pool.tile([128, kk], f32, tag=f"s1{coord}")
                            nc.vector.tensor_sub(s1[:], sm[:], s2[:])
                            w3[coord] = (s0, s1, s2)

                        w9 = pool.tile([128, 9 * kk], f16, tag="w9")
                        w9v = w9[:].rearrange("p (j k) -> p j k", j=9)
                        for jx in range(3):
                            for jy in range(3):
                                nc.vector.tensor_mul(
                                    w9v[:, jx * 3 + jy, :],
                                    w3["x"][jx][:], w3["y"][jy][:])
                        w9b = (w9[:].rearrange("p (j k) -> p j k", j=9)
                               .transpose([0, 2, 1]).unsqueeze(3)
                               .broadcast_to([128, kk, 9, C]))
                        lv = p3[:, :, off:off + 72].rearrange(
                            "p k (j c) -> p k j c", c=C)
                        nc.vector.tensor_mul(lv, w9b, lv)
                        o = off
                        nc.vector.tensor_add(pv[:, :, o:o + 24],
                                             pv[:, :, o:o + 24],
                                             pv[:, :, o + 24:o + 48])
                        nc.vector.tensor_add(pv[:, :, o:o + 24],
                                             pv[:, :, o:o + 24],
                                             pv[:, :, o + 48:o + 72])
                        nc.vector.tensor_add(pv[:, :, o:o + 8],
                                             pv[:, :, o:o + 8],
                                             pv[:, :, o + 8:o + 16])
                        nc.vector.tensor_add(pv[:, :, o:o + 8],
                                             pv[:, :, o:o + 8],
                                             pv[:, :, o + 16:o + 24])
                        nc.vector.tensor_add(acc[:], acc[:], pv[:, :, o:o + 8])
                        off += 72

                    accv = acc[:].rearrange("p (k c) -> p k c", c=C)
                    stage = pool.tile([128, kk * C], f32, tag="stage")
                    stv = stage[:].rearrange("p (c k) -> p c k", c=C)
                    for c in range(C):
                        nc.vector.tensor_copy(stv[:, c, :], accv[:, :, c])
                        nc.sync.dma_start(
                            out_d.ap()[c, r0:r0 + 128, w0:w0 + kk],
                            stv[:, c, :])
    nc.compile()
    return nc


def _get_nc(key, *args):
    if key not in _CACHED:
        _CACHED[key] = _build_nc(*args)
    return _CACHED[key]


def kernel(uv_tensor, iter_nr, tex0, tex1, tex2, tex3):
    from concourse import bass_utils

    bass_utils.upload_artifacts = lambda tmpdir: "local://" + tmpdir

    uv = np.asarray(uv_tensor, dtype=np.float32)
    assert uv.shape == (1, 2, FULL_H, FULL_W), uv.shape
    tbl = _build_mega_table(tex0, tex1, tex2, tex3)

    nc = _get_nc("full", ROWS, FULL_W, K)

    in_maps = []
    for i in range(N_CORES):
        r0 = i * ROWS
        in_maps.append({
            "u": np.ascontiguousarray(uv[0, 0, r0:r0 + ROWS, :]),
            "v": np.ascontiguousarray(uv[0, 1, r0:r0 + ROWS, :]),
            "tbl": tbl,
        })

    res = bass_utils.run_bass_kernel_spmd(
        nc, in_maps, core_ids=list(range(N_CORES)))
    globals()["_LAST_RES"] = res
    out = np.concatenate(
        [res.results[i]["out"][None] for i in range(N_CORES)], axis=2)
    return out.astype(np.float32)



# revision 6
# speedup vs baseline: 1.4649x; 1.4649x over previous
"""Trainium2 Bass kernel for nn_DeferredRender (4-level bilinear grid_sample sum).

Bilinear-coefficient mega-entry design
--------------------------------------
Key by the "virtual half-cell" h = floor(u*2048) per axis. For every level L
(texture width W_L = 1024 >> L), the cell x0_L = floor(u*W_L - 0.5) is a pure
function of h:  x0_L = (h - 2^L) >> (L+1)  (exact dyadic argument), and the
fraction decomposes as  fx_L = (X + mx_L) * 2^-(L+1)  with X = u*2048 - h and
mx_L = (h - 2^L) mod 2^(L+1)  -- a function of h alone.

Writing each level's bilinear sample as a + fx*dx + fy*dy + fx*fy*dxy and
substituting, the ENTIRE 4-level sum collapses to

    out[c] = A[c] + X*B[c] + Y*C[c] + (X*Y)*D[c]

with A,B,C,D four 8-channel vectors precomputed per (hy, hx) key on the host
(the mx/my cross terms fold into A,B,C). One 64-byte gather per pixel and six
vector ops per tile replace all per-level weight machinery. The algebra is an
exact reparameterization, valid for either tie-break of h = round(u*2048-0.5),
since X is computed against the same h.

Device kernel (per core, 256 of 2048 rows, H-sharded 8 ways): per [128 x KK]
pixel block, compute (hx, hy, X, Y, XY, idx) on DVE/ACT, fetch one 64B entry
per pixel via SWDGE indirect DMA ([128,1] indices per instruction -- the only
HW-supported form -- rotated across NQ SWDGE queues), then 3 broadcast-muls +
3 adds, store channels-last fp16; host transposes.
"""

import numpy as np

C = 8
FULL_H = 2048
FULL_W = 2048
N_CORES = 8
ROWS = FULL_H // N_CORES  # 256
KK = 512    # pixels per block column chunk
NQ = 4      # SWDGE queues to rotate indirect DMAs across

G = 2048    # half-cell grid
ENT = 32    # fp16 elems per entry: A(8) B(8) C(8) D(8)

_CACHED = {}


def _build_coeff_table(tex0, tex1, tex2, tex3):
    """[G*G, 32] fp16: per (hy, hx) the A,B,C,D 8-channel coefficients."""
    texs = [np.asarray(t, np.float32) for t in (tex0, tex1, tex2, tex3)]
    h = np.arange(G)
    A = np.zeros((G, G, C), np.float32)
    B = np.zeros((G, G, C), np.float32)
    Cc = np.zeros((G, G, C), np.float32)
    D = np.zeros((G, G, C), np.float32)
    for L, tex in enumerate(texs):
        W = tex.shape[2]  # == tex.shape[1]
        two = 1 << L
        den = 1 << (L + 1)
        s = 1.0 / den
        x0 = (h - two) >> (L + 1)          # [G] in [-1, W-1]
        m = (h - two) - (x0 << (L + 1))    # [G] in [0, den)
        ms = (m * s).astype(np.float32)    # [G]
        t = tex.transpose(1, 2, 0)         # [H, W, C]

        def rows(yi):
            v = (yi >= 0) & (yi < W)
            return t[np.clip(yi, 0, W - 1)] * v[:, None, None]

        r0 = rows(x0)          # [G, W, C] rows y0 (zero OOB)
        r1 = rows(x0 + 1)      # rows y0+1

        def cols(r, xi):
            v = (xi >= 0) & (xi < W)
            return r[:, np.clip(xi, 0, W - 1)] * v[None, :, None]

        c00 = cols(r0, x0)
        c10 = cols(r0, x0 + 1)
        c01 = cols(r1, x0)
        c11 = cols(r1, x0 + 1)
        del r0, r1
        dx = c10 - c00
        dy = c01 - c00
        dxy = c11 - c10 - c01 + c00
        del c10, c01, c11
        a = c00
        del c00
        msx = ms[None, :, None]
        msy = ms[:, None, None]
        A += a + msx * dx + msy * dy + (msy * msx) * dxy
        B += s * (dx + msy * dxy)
        Cc += s * (dy + msx * dxy)
        D += (s * s) * dxy
        del a, dx, dy, dxy
    out = np.empty((G, G, 4, C), np.float16)
    out[:, :, 0] = A
    out[:, :, 1] = B
    out[:, :, 2] = Cc
    out[:, :, 3] = D
    return np.ascontiguousarray(out.reshape(G * G, 4 * C))


def _build_nc(rows, width):
    import concourse.bacc as bacc
    import concourse.bass as bass
    import concourse.mybir as mybir
    import concourse.tile as tile

    f32 = mybir.dt.float32
    f16 = mybir.dt.float16
    i32 = mybir.dt.int32
    Copy = mybir.ActivationFunctionType.Copy
    MUL = mybir.AluOpType.mult
    ADD = mybir.AluOpType.add
    SUB = mybir.AluOpType.subtract

    nc = bacc.Bacc("TRN2", target_bir_lowering=False, debug=False,
                   num_devices=N_CORES, num_swdge_queues=NQ)
    u_d = nc.dram_tensor("u", [rows, width], f32, kind="ExternalInput")
    v_d = nc.dram_tensor("v", [rows, width], f32, kind="ExternalInput")
    tbl_d = nc.dram_tensor("tbl", [G * G, ENT], f16, kind="ExternalInput")
    out_d = nc.dram_tensor("out", [rows, width * C], f16,
                           kind="ExternalOutput")

    with tile.TileContext(nc) as tc:
        with tc.tile_pool(name="main", bufs=2) as pool:
            for r0 in range(0, rows, 128):
                for w0 in range(0, width, KK):
                    u_t = pool.tile([128, KK], f32, tag="u")
                    v_t = pool.tile([128, KK], f32, tag="v")
                    nc.sync.dma_start(u_t[:], u_d.ap()[r0:r0 + 128,
                                                       w0:w0 + KK])
                    nc.sync.dma_start(v_t[:], v_d.ap()[r0:r0 + 128,
                                                       w0:w0 + KK])

                    def cell(src, tagp):
                        """h = round(u*2048 - 0.5); X = u*2048 - h."""
                        s2 = pool.tile([128, KK], f32, tag=f"s2{tagp}")
                        nc.scalar.activation(s2[:], src[:], Copy,
                                             bias=-0.5, scale=float(G))
                        hi = pool.tile([128, KK], i32, tag=f"hi{tagp}")
                        nc.vector.tensor_copy(hi[:], s2[:])
                        nc.vector.tensor_scalar_max(hi[:], hi[:], 0)
                        hf = pool.tile([128, KK], f32, tag=f"hf{tagp}")
                        nc.vector.tensor_copy(hf[:], hi[:])
                        X = pool.tile([128, KK], f32, tag=f"X{tagp}")
                        nc.vector.scalar_tensor_tensor(
                            out=X[:], in0=s2[:], scalar=0.5, in1=hf[:],
                            op0=ADD, op1=SUB)
                        return hi, X

                    hxi, X = cell(u_t, "x")
                    hyi, Y = cell(v_t, "y")

                    XY = pool.tile([128, KK], f32, tag="XY")
                    nc.vector.tensor_mul(XY[:], X[:], Y[:])
                    idx = pool.tile([128, KK], i32, tag="idx")
                    nc.vector.scalar_tensor_tensor(
                        out=idx[:], in0=hyi[:], scalar=G, in1=hxi[:],
                        op0=MUL, op1=ADD)

                    def indirect_q(out_ap, in_ap_full, off_ap, qname):
                        """indirect_dma_start clone with a selectable SWDGE
                        queue (the library hardcodes qPoolDynamic)."""
                        g = nc.gpsimd
                        out_l = g.lower_ap_dma(out_ap, for_indirect_dma=True)
                        in_l = g.lower_ap_dma(in_ap_full,
                                              for_indirect_dma=True)
                        assert len(in_l) == 1 and len(out_l) == 1
                        off_l = g.lower_ap_dma(off_ap)
                        assert len(off_l) == 1
                        in_l.append(off_l[0])
                        coef = 1
                        for d in in_ap_full.shape[1:]:
                            coef *= d
                        in_l[0].dynamic_ap_info = mybir.DynamicAccessPatternInfo(
                            c=0,
                            actual_ap=out_ap.ap,
                            indirect_dim_max_index=in_ap_full.shape[0],
                            offset_expr=[
                                mybir.DynamicAccessPatternOffsetExpr(
                                    coef=coef,
                                    aff_expr=mybir.DynamicAccessPatternOffsetExprAffExpr(
                                        kind="IndirectArgId", arg_id=1),
                                )
                            ],
                        )
                        return g.add_instruction(
                            mybir.InstDMACopy(
                                name=g.bass.get_next_instruction_name(),
                                queue=qname,
                                mode="Copy",
                                ins=in_l,
                                outs=out_l,
                                oob_is_err=True,
                                cce_op=mybir.AluOpType.bypass,
                            ))

                    patch = pool.tile([128, KK * ENT], f16, tag="patch")
                    p3 = patch[:].rearrange("p (k e) -> p k e", e=ENT)
                    for k in range(KK):
                        qname = f"qPoolDynamic{(k % NQ) or ''}"
                        indirect_q(p3[:, k, :], tbl_d.ap(),
                                   idx[:, k:k + 1], qname)

                    pv = patch[:].rearrange("p (k j c) -> p k j c", j=4, c=C)
                    m1 = pool.tile([128, KK * C], f16, tag="m1")
                    m2 = pool.tile([128, KK * C], f16, tag="m2")
                    m3 = pool.tile([128, KK * C], f16, tag="m3")
                    m1v = m1[:].rearrange("p (k c) -> p k c", c=C)
                    m2v = m2[:].rearrange("p (k c) -> p k c", c=C)
                    m3v = m3[:].rearrange("p (k c) -> p k c", c=C)
                    Xb = X[:].unsqueeze(2).broadcast_to([128, KK, C])
                    Yb = Y[:].unsqueeze(2).broadcast_to([128, KK, C])
                    XYb = XY[:].unsqueeze(2).broadcast_to([128, KK, C])
                    nc.vector.tensor_mul(m1v, Xb, pv[:, :, 1, :])
                    nc.vector.tensor_mul(m2v, Yb, pv[:, :, 2, :])
                    nc.vector.tensor_mul(m3v, XYb, pv[:, :, 3, :])
                    # S1 = A + M1 ; S2 = M2 + M3 ; OUT = S1 + S2
                    nc.vector.tensor_add(m1v, m1v, pv[:, :, 0, :])
                    nc.vector.tensor_add(m2v, m2v, m3v)
                    ot = pool.tile([128, KK * C], f16, tag="ot")
                    nc.vector.tensor_add(ot[:], m1[:], m2[:])
                    nc.sync.dma_start(
                        out_d.ap()[r0:r0 + 128,
                                   w0 * C:(w0 + KK) * C], ot[:])
    nc.compile()
    return nc


def _get_nc(key, *args):
    if key not in _CACHED:
        _CACHED[key] = _build_nc(*args)
    return _CACHED[key]


def kernel(uv_tensor, iter_nr, tex0, tex1, tex2, tex3):
    from concourse import bass_utils

    bass_utils.upload_artifacts = lambda tmpdir: "local://" + tmpdir

    uv = np.asarray(uv_tensor, dtype=np.float32)
    assert uv.shape == (1, 2, FULL_H, FULL_W), uv.shape
    tbl = _build_coeff_table(tex0, tex1, tex2, tex3)

    nc = _get_nc("full", ROWS, FULL_W)

    in_maps = []
    for i in range(N_CORES):
        r0 = i * ROWS
        in_maps.append({
            "u": np.ascontiguousarray(uv[0, 0, r0:r0 + ROWS, :]),
            "v": np.ascontiguousarray(uv[0, 1, r0:r0 + ROWS, :]),
            "tbl": tbl,
        })

    res = bass_utils.run_bass_kernel_spmd(
        nc, in_maps, core_ids=list(range(N_CORES)))
    globals()["_LAST_RES"] = res
    parts = []
    for i in range(N_CORES):
        o = res.results[i]["out"].reshape(ROWS, FULL_W, C)
        parts.append(np.transpose(o, (2, 0, 1)).astype(np.float32))
    out = np.concatenate(parts, axis=1)[None]
    return out


# revision 10
# speedup vs baseline: 1.4797x; 1.0101x over previous
"""Trainium2 Bass kernel for nn_DeferredRender (4-level bilinear grid_sample sum).

Bilinear-coefficient mega-entry design
--------------------------------------
Key by the "virtual half-cell" h = floor(u*2048) per axis. For every level L
(texture width W_L = 1024 >> L), the cell x0_L = floor(u*W_L - 0.5) is a pure
function of h:  x0_L = (h - 2^L) >> (L+1)  (exact dyadic argument), and the
fraction decomposes as  fx_L = (X + mx_L) * 2^-(L+1)  with X = u*2048 - h and
mx_L = (h - 2^L) mod 2^(L+1)  -- a function of h alone.

Writing each level's bilinear sample as a + fx*dx + fy*dy + fx*fy*dxy and
substituting, the ENTIRE 4-level sum collapses to

    out[c] = A[c] + X*B[c] + Y*C[c] + (X*Y)*D[c]

with A,B,C,D four 8-channel vectors precomputed per (hy, hx) key on the host
(the mx/my cross terms fold into A,B,C). One 64-byte gather per pixel and six
vector ops per tile replace all per-level weight machinery. The algebra is an
exact reparameterization, valid for either tie-break of h = round(u*2048-0.5),
since X is computed against the same h.

Device kernel (per core, 256 of 2048 rows, H-sharded 8 ways): per [128 x KK]
pixel block, compute (hx, hy, X, Y, XY, idx) on DVE/ACT, fetch one 64B entry
per pixel via SWDGE indirect DMA ([128,1] indices per instruction -- the only
HW-supported form -- rotated across NQ SWDGE queues), then 3 broadcast-muls +
3 adds, store channels-last fp16; host transposes.
"""

import numpy as np

C = 8
FULL_H = 2048
FULL_W = 2048
N_CORES = 8
ROWS = FULL_H // N_CORES  # 256
KK = 128    # pixels per block column chunk
NQ = 4      # SWDGE queues to rotate indirect DMAs across
BUFS = 4    # tile-pool double-buffering depth

G = 2048    # half-cell grid
ENT = 32    # fp16 elems per entry: A(8) B(8) C(8) D(8)

_CACHED = {}


def _build_coeff_table(tex0, tex1, tex2, tex3):
    """[G*G, 32] fp16: per (hy, hx) the A,B,C,D 8-channel coefficients."""
    texs = [np.asarray(t, np.float32) for t in (tex0, tex1, tex2, tex3)]
    h = np.arange(G)
    A = np.zeros((G, G, C), np.float32)
    B = np.zeros((G, G, C), np.float32)
    Cc = np.zeros((G, G, C), np.float32)
    D = np.zeros((G, G, C), np.float32)
    for L, tex in enumerate(texs):
        W = tex.shape[2]  # == tex.shape[1]
        two = 1 << L
        den = 1 << (L + 1)
        s = 1.0 / den
        x0 = (h - two) >> (L + 1)          # [G] in [-1, W-1]
        m = (h - two) - (x0 << (L + 1))    # [G] in [0, den)
        ms = (m * s).astype(np.float32)    # [G]
        t = tex.transpose(1, 2, 0)         # [H, W, C]

        def rows(yi):
            v = (yi >= 0) & (yi < W)
            return t[np.clip(yi, 0, W - 1)] * v[:, None, None]

        r0 = rows(x0)          # [G, W, C] rows y0 (zero OOB)
        r1 = rows(x0 + 1)      # rows y0+1

        def cols(r, xi):
            v = (xi >= 0) & (xi < W)
            return r[:, np.clip(xi, 0, W - 1)] * v[None, :, None]

        c00 = cols(r0, x0)
        c10 = cols(r0, x0 + 1)
        c01 = cols(r1, x0)
        c11 = cols(r1, x0 + 1)
        del r0, r1
        dx = c10 - c00
        dy = c01 - c00
        dxy = c11 - c10 - c01 + c00
        del c10, c01, c11
        a = c00
        del c00
        msx = ms[None, :, None]
        msy = ms[:, None, None]
        A += a + msx * dx + msy * dy + (msy * msx) * dxy
        B += s * (dx + msy * dxy)
        Cc += s * (dy + msx * dxy)
        D += (s * s) * dxy
        del a, dx, dy, dxy
    out = np.empty((G, G, 4, C), np.float16)
    out[:, :, 0] = A
    out[:, :, 1] = B
    out[:, :, 2] = Cc
    out[:, :, 3] = D
    return np.ascontiguousarray(out.reshape(G * G, 4 * C))


def _build_nc(rows, width):
    import concourse.bacc as bacc
    import concourse.bass as bass
    import concourse.mybir as mybir
    import concourse.tile as tile

    f32 = mybir.dt.float32
    f16 = mybir.dt.float16
    i32 = mybir.dt.int32
    Copy = mybir.ActivationFunctionType.Copy
    MUL = mybir.AluOpType.mult
    ADD = mybir.AluOpType.add
    SUB = mybir.AluOpType.subtract

    nc = bacc.Bacc("TRN2", target_bir_lowering=False, debug=False,
                   num_devices=N_CORES, num_swdge_queues=NQ)
    u_d = nc.dram_tensor("u", [rows, width], f32, kind="ExternalInput")
    v_d = nc.dram_tensor("v", [rows, width], f32, kind="ExternalInput")
    tbl_d = nc.dram_tensor("tbl", [G * G, ENT], f16, kind="ExternalInput")
    out_d = nc.dram_tensor("out", [rows, width * C], f16,
                           kind="ExternalOutput")

    with tile.TileContext(nc) as tc:
        with tc.tile_pool(name="main", bufs=BUFS) as pool:
            for r0 in range(0, rows, 128):
                for w0 in range(0, width, KK):
                    u_t = pool.tile([128, KK], f32, tag="u")
                    v_t = pool.tile([128, KK], f32, tag="v")
                    nc.sync.dma_start(u_t[:], u_d.ap()[r0:r0 + 128,
                                                       w0:w0 + KK])
                    nc.sync.dma_start(v_t[:], v_d.ap()[r0:r0 + 128,
                                                       w0:w0 + KK])

                    def cell(src, tagp):
                        """h = round(u*2048 - 0.5); X = u*2048 - h."""
                        s2 = pool.tile([128, KK], f32, tag=f"s2{tagp}")
                        nc.scalar.activation(s2[:], src[:], Copy,
                                             bias=-0.5, scale=float(G))
                        hi = pool.tile([128, KK], i32, tag=f"hi{tagp}")
                        nc.vector.tensor_copy(hi[:], s2[:])
                        nc.vector.tensor_scalar_max(hi[:], hi[:], 0)
                        hf = pool.tile([128, KK], f32, tag=f"hf{tagp}")
                        nc.vector.tensor_copy(hf[:], hi[:])
                        X = pool.tile([128, KK], f32, tag=f"X{tagp}")
                        nc.vector.scalar_tensor_tensor(
                            out=X[:], in0=s2[:], scalar=0.5, in1=hf[:],
                            op0=ADD, op1=SUB)
                        return hi, X

                    hxi, X = cell(u_t, "x")
                    hyi, Y = cell(v_t, "y")

                    XY = pool.tile([128, KK], f32, tag="XY")
                    nc.vector.tensor_mul(XY[:], X[:], Y[:])
                    # idx pre-scaled to element units (coef=1 in the DMA)
                    idx = pool.tile([128, KK], i32, tag="idx")
                    nc.vector.scalar_tensor_tensor(
                        out=idx[:], in0=hyi[:], scalar=G, in1=hxi[:],
                        op0=MUL, op1=ADD)
                    nc.vector.tensor_scalar_mul(idx[:], idx[:], ENT)

                    def indirect_q(out_ap, in_ap_full, off_ap, qname):
                        """indirect_dma_start clone with a selectable SWDGE
                        queue (the library hardcodes qPoolDynamic)."""
                        g = nc.gpsimd
                        out_l = g.lower_ap_dma(out_ap, for_indirect_dma=True)
                        in_l = g.lower_ap_dma(in_ap_full,
                                              for_indirect_dma=True)
                        assert len(in_l) == 1 and len(out_l) == 1
                        off_l = g.lower_ap_dma(off_ap)
                        assert len(off_l) == 1
                        in_l.append(off_l[0])
                        coef = 1  # idx is pre-scaled to element units on DVE
                        in_l[0].dynamic_ap_info = mybir.DynamicAccessPatternInfo(
                            c=0,
                            actual_ap=out_ap.ap,
                            indirect_dim_max_index=in_ap_full.shape[0],
                            offset_expr=[
                                mybir.DynamicAccessPatternOffsetExpr(
                                    coef=coef,
                                    aff_expr=mybir.DynamicAccessPatternOffsetExprAffExpr(
                                        kind="IndirectArgId", arg_id=1),
                                )
                            ],
                        )
                        return g.add_instruction(
                            mybir.InstDMACopy(
                                name=g.bass.get_next_instruction_name(),
                                queue=qname,
                                mode="Copy",
                                ins=in_l,
                                outs=out_l,
                                oob_is_err=True,
                                cce_op=mybir.AluOpType.bypass,
                            ))

                    patch = pool.tile([128, KK * ENT], f16, tag="patch")
                    p3 = patch[:].rearrange("p (k e) -> p k e", e=ENT)
                    for k in range(KK):
                        qname = f"qPoolDynamic{(k % NQ) or ''}"
                        indirect_q(p3[:, k, :], tbl_d.ap(),
                                   idx[:, k:k + 1], qname)

                    pv = patch[:].rearrange("p (k j c) -> p k j c", j=4, c=C)
                    m1 = pool.tile([128, KK * C], f16, tag="m1")
                    m2 = pool.tile([128, KK * C], f16, tag="m2")
                    m3 = pool.tile([128, KK * C], f16, tag="m3")
                    m1v = m1[:].rearrange("p (k c) -> p k c", c=C)
                    m2v = m2[:].rearrange("p (k c) -> p k c", c=C)
                    m3v = m3[:].rearrange("p (k c) -> p k c", c=C)
                    Xb = X[:].unsqueeze(2).broadcast_to([128, KK, C])
                    Yb = Y[:].unsqueeze(2).broadcast_to([128, KK, C])
                    XYb = XY[:].unsqueeze(2).broadcast_to([128, KK, C])
                    nc.vector.tensor_mul(m1v, Xb, pv[:, :, 1, :])
                    nc.vector.tensor_mul(m2v, Yb, pv[:, :, 2, :])
                    nc.vector.tensor_mul(m3v, XYb, pv[:, :, 3, :])
                    # S1 = A + M1 ; S2 = M2 + M3 ; OUT = S1 + S2
                    nc.vector.tensor_add(m1v, m1v, pv[:, :, 0, :])
                    nc.vector.tensor_add(m2v, m2v, m3v)
                    ot = pool.tile([128, KK * C], f16, tag="ot")
                    nc.vector.tensor_add(ot[:], m1[:], m2[:])
                    nc.sync.dma_start(
                        out_d.ap()[r0:r0 + 128,
                                   w0 * C:(w0 + KK) * C], ot[:])
    nc.compile()
    return nc


def _get_nc(key, *args):
    if key not in _CACHED:
        _CACHED[key] = _build_nc(*args)
    return _CACHED[key]


def kernel(uv_tensor, iter_nr, tex0, tex1, tex2, tex3):
    from concourse import bass_utils

    bass_utils.upload_artifacts = lambda tmpdir: "local://" + tmpdir

    uv = np.asarray(uv_tensor, dtype=np.float32)
    assert uv.shape == (1, 2, FULL_H, FULL_W), uv.shape
    tbl = _build_coeff_table(tex0, tex1, tex2, tex3)

    nc = _get_nc("full", ROWS, FULL_W)

    in_maps = []
    for i in range(N_CORES):
        r0 = i * ROWS
        in_maps.append({
            "u": np.ascontiguousarray(uv[0, 0, r0:r0 + ROWS, :]),
            "v": np.ascontiguousarray(uv[0, 1, r0:r0 + ROWS, :]),
            "tbl": tbl,
        })

    res = bass_utils.run_bass_kernel_spmd(
        nc, in_maps, core_ids=list(range(N_CORES)))
    globals()["_LAST_RES"] = res
    parts = []
    for i in range(N_CORES):
        o = res.results[i]["out"].reshape(ROWS, FULL_W, C)
        parts.append(np.transpose(o, (2, 0, 1)).astype(np.float32))
    out = np.concatenate(parts, axis=1)[None]
    return out


# revision 11
# speedup vs baseline: 1.4891x; 1.0063x over previous
"""Trainium2 Bass kernel for nn_DeferredRender (4-level bilinear grid_sample sum).

Bilinear-coefficient mega-entry design
--------------------------------------
Key by the "virtual half-cell" h = floor(u*2048) per axis. For every level L
(texture width W_L = 1024 >> L), the cell x0_L = floor(u*W_L - 0.5) is a pure
function of h:  x0_L = (h - 2^L) >> (L+1)  (exact dyadic argument), and the
fraction decomposes as  fx_L = (X + mx_L) * 2^-(L+1)  with X = u*2048 - h and
mx_L = (h - 2^L) mod 2^(L+1)  -- a function of h alone.

Writing each level's bilinear sample as a + fx*dx + fy*dy + fx*fy*dxy and
substituting, the ENTIRE 4-level sum collapses to

    out[c] = A[c] + X*B[c] + Y*C[c] + (X*Y)*D[c]

with A,B,C,D four 8-channel vectors precomputed per (hy, hx) key on the host
(the mx/my cross terms fold into A,B,C). One 64-byte gather per pixel and six
vector ops per tile replace all per-level weight machinery. The algebra is an
exact reparameterization, valid for either tie-break of h = round(u*2048-0.5),
since X is computed against the same h.

Device kernel (per core, 256 of 2048 rows, H-sharded 8 ways): per [128 x KK]
pixel block, compute (hx, hy, X, Y, XY, idx) on DVE/ACT, fetch one 64B entry
per pixel via SWDGE indirect DMA ([128,1] indices per instruction -- the only
HW-supported form -- rotated across NQ SWDGE queues), then 3 broadcast-muls +
3 adds, store channels-last fp16; host transposes.
"""

import numpy as np

C = 8
FULL_H = 2048
FULL_W = 2048
N_CORES = 8
ROWS = FULL_H // N_CORES  # 256
KK = 128    # pixels per block column chunk
NQ = 4      # SWDGE queues to rotate indirect DMAs across
BUFS = 4    # tile-pool double-buffering depth

G = 2048    # half-cell grid
ENT = 32    # fp16 elems per entry: A(8) B(8) C(8) D(8)

_CACHED = {}


def _build_coeff_table(tex0, tex1, tex2, tex3):
    """[G*G, 32] fp16: per (hy, hx) the A,B,C,D 8-channel coefficients."""
    texs = [np.asarray(t, np.float32) for t in (tex0, tex1, tex2, tex3)]
    h = np.arange(G)
    A = np.zeros((G, G, C), np.float32)
    B = np.zeros((G, G, C), np.float32)
    Cc = np.zeros((G, G, C), np.float32)
    D = np.zeros((G, G, C), np.float32)
    for L, tex in enumerate(texs):
        W = tex.shape[2]  # == tex.shape[1]
        two = 1 << L
        den = 1 << (L + 1)
        s = 1.0 / den
        x0 = (h - two) >> (L + 1)          # [G] in [-1, W-1]
        m = (h - two) - (x0 << (L + 1))    # [G] in [0, den)
        ms = (m * s).astype(np.float32)    # [G]
        t = tex.transpose(1, 2, 0)         # [H, W, C]

        def rows(yi):
            v = (yi >= 0) & (yi < W)
            return t[np.clip(yi, 0, W - 1)] * v[:, None, None]

        r0 = rows(x0)          # [G, W, C] rows y0 (zero OOB)
        r1 = rows(x0 + 1)      # rows y0+1

        def cols(r, xi):
            v = (xi >= 0) & (xi < W)
            return r[:, np.clip(xi, 0, W - 1)] * v[None, :, None]

        c00 = cols(r0, x0)
        c10 = cols(r0, x0 + 1)
        c01 = cols(r1, x0)
        c11 = cols(r1, x0 + 1)
        del r0, r1
        dx = c10 - c00
        dy = c01 - c00
        dxy = c11 - c10 - c01 + c00
        del c10, c01, c11
        a = c00
        del c00
        msx = ms[None, :, None]
        msy = ms[:, None, None]
        A += a + msx * dx + msy * dy + (msy * msx) * dxy
        B += s * (dx + msy * dxy)
        Cc += s * (dy + msx * dxy)
        D += (s * s) * dxy
        del a, dx, dy, dxy
    out = np.empty((G, G, 4, C), np.float16)
    out[:, :, 0] = A
    out[:, :, 1] = B
    out[:, :, 2] = Cc
    out[:, :, 3] = D
    return np.ascontiguousarray(out.reshape(G * G, 4 * C))


def _build_nc(rows, width):
    import concourse.bacc as bacc
    import concourse.bass as bass
    import concourse.mybir as mybir
    import concourse.tile as tile

    f32 = mybir.dt.float32
    f16 = mybir.dt.float16
    i32 = mybir.dt.int32
    Copy = mybir.ActivationFunctionType.Copy
    MUL = mybir.AluOpType.mult
    ADD = mybir.AluOpType.add
    SUB = mybir.AluOpType.subtract

    nc = bacc.Bacc("TRN2", target_bir_lowering=False, debug=False,
                   num_devices=N_CORES, num_swdge_queues=NQ,
                   dynamic_dma_scratch_size=65536)
    u_d = nc.dram_tensor("u", [rows, width], f32, kind="ExternalInput")
    v_d = nc.dram_tensor("v", [rows, width], f32, kind="ExternalInput")
    tbl_d = nc.dram_tensor("tbl", [G * G, ENT], f16, kind="ExternalInput")
    out_d = nc.dram_tensor("out", [rows, width * C], f16,
                           kind="ExternalOutput")

    with tile.TileContext(nc) as tc:
        with tc.tile_pool(name="main", bufs=BUFS) as pool:
            for r0 in range(0, rows, 128):
                for w0 in range(0, width, KK):
                    u_t = pool.tile([128, KK], f32, tag="u")
                    v_t = pool.tile([128, KK], f32, tag="v")
                    nc.sync.dma_start(u_t[:], u_d.ap()[r0:r0 + 128,
                                                       w0:w0 + KK])
                    nc.sync.dma_start(v_t[:], v_d.ap()[r0:r0 + 128,
                                                       w0:w0 + KK])

                    def cell(src, tagp):
                        """h = round(u*2048 - 0.5); X = u*2048 - h."""
                        s2 = pool.tile([128, KK], f32, tag=f"s2{tagp}")
                        nc.scalar.activation(s2[:], src[:], Copy,
                                             bias=-0.5, scale=float(G))
                        hi = pool.tile([128, KK], i32, tag=f"hi{tagp}")
                        nc.vector.tensor_copy(hi[:], s2[:])
                        nc.vector.tensor_scalar_max(hi[:], hi[:], 0)
                        hf = pool.tile([128, KK], f32, tag=f"hf{tagp}")
                        nc.vector.tensor_copy(hf[:], hi[:])
                        X = pool.tile([128, KK], f32, tag=f"X{tagp}")
                        nc.vector.scalar_tensor_tensor(
                            out=X[:], in0=s2[:], scalar=0.5, in1=hf[:],
                            op0=ADD, op1=SUB)
                        return hi, X

                    hxi, X = cell(u_t, "x")
                    hyi, Y = cell(v_t, "y")

                    XY = pool.tile([128, KK], f32, tag="XY")
                    nc.vector.tensor_mul(XY[:], X[:], Y[:])
                    # idx pre-scaled to element units (coef=1 in the DMA)
                    idx = pool.tile([128, KK], i32, tag="idx")
                    nc.vector.scalar_tensor_tensor(
                        out=idx[:], in0=hyi[:], scalar=G, in1=hxi[:],
                        op0=MUL, op1=ADD)
                    nc.vector.tensor_scalar_mul(idx[:], idx[:], ENT)

                    def indirect_q(out_ap, in_ap_full, off_ap, qname):
                        """indirect_dma_start clone with a selectable SWDGE
                        queue (the library hardcodes qPoolDynamic)."""
                        g = nc.gpsimd
                        out_l = g.lower_ap_dma(out_ap, for_indirect_dma=True)
                        in_l = g.lower_ap_dma(in_ap_full,
                                              for_indirect_dma=True)
                        assert len(in_l) == 1 and len(out_l) == 1
                        off_l = g.lower_ap_dma(off_ap)
                        assert len(off_l) == 1
                        in_l.append(off_l[0])
                        coef = 1  # idx is pre-scaled to element units on DVE
                        in_l[0].dynamic_ap_info = mybir.DynamicAccessPatternInfo(
                            c=0,
                            actual_ap=out_ap.ap,
                            indirect_dim_max_index=in_ap_full.shape[0],
                            offset_expr=[
                                mybir.DynamicAccessPatternOffsetExpr(
                                    coef=coef,
                                    aff_expr=mybir.DynamicAccessPatternOffsetExprAffExpr(
                                        kind="IndirectArgId", arg_id=1),
                                )
                            ],
                        )
                        return g.add_instruction(
                            mybir.InstDMACopy(
                                name=g.bass.get_next_instruction_name(),
                                queue=qname,
                                mode="Copy",
                                ins=in_l,
                                outs=out_l,
                                oob_is_err=True,
                                cce_op=mybir.AluOpType.bypass,
                            ))

                    patch = pool.tile([128, KK * ENT], f16, tag="patch")
                    p3 = patch[:].rearrange("p (k e) -> p k e", e=ENT)
                    for k in range(KK):
                        qname = f"qPoolDynamic{(k % NQ) or ''}"
                        indirect_q(p3[:, k, :], tbl_d.ap(),
                                   idx[:, k:k + 1], qname)

                    pv = patch[:].rearrange("p (k j c) -> p k j c", j=4, c=C)
                    m1 = pool.tile([128, KK * C], f16, tag="m1")
                    m2 = pool.tile([128, KK * C], f16, tag="m2")
                    m3 = pool.tile([128, KK * C], f16, tag="m3")
                    m1v = m1[:].rearrange("p (k c) -> p k c", c=C)
                    m2v = m2[:].rearrange("p (k c) -> p k c", c=C)
                    m3v = m3[:].rearrange("p (k c) -> p k c", c=C)
                    Xb = X[:].unsqueeze(2).broadcast_to([128, KK, C])
                    Yb = Y[:].unsqueeze(2).broadcast_to([128, KK, C])
                    XYb = XY[:].unsqueeze(2).broadcast_to([128, KK, C])
                    nc.vector.tensor_mul(m1v, Xb, pv[:, :, 1, :])
                    nc.vector.tensor_mul(m2v, Yb, pv[:, :, 2, :])
                    nc.vector.tensor_mul(m3v, XYb, pv[:, :, 3, :])
                    # S1 = A + M1 ; S2 = M2 + M3 ; OUT = S1 + S2
                    nc.vector.tensor_add(m1v, m1v, pv[:, :, 0, :])
                    nc.vector.tensor_add(m2v, m2v, m3v)
                    ot = pool.tile([128, KK * C], f16, tag="ot")
                    nc.vector.tensor_add(ot[:], m1[:], m2[:])
                    nc.sync.dma_start(
                        out_d.ap()[r0:r0 + 128,
                                   w0 * C:(w0 + KK) * C], ot[:])
    nc.compile()
    return nc


def _get_nc(key, *args):
    if key not in _CACHED:
        _CACHED[key] = _build_nc(*args)
    return _CACHED[key]


def kernel(uv_tensor, iter_nr, tex0, tex1, tex2, tex3):
    from concourse import bass_utils

    bass_utils.upload_artifacts = lambda tmpdir: "local://" + tmpdir

    uv = np.asarray(uv_tensor, dtype=np.float32)
    assert uv.shape == (1, 2, FULL_H, FULL_W), uv.shape
    tbl = _build_coeff_table(tex0, tex1, tex2, tex3)

    nc = _get_nc("full", ROWS, FULL_W)

    in_maps = []
    for i in range(N_CORES):
        r0 = i * ROWS
        in_maps.append({
            "u": np.ascontiguousarray(uv[0, 0, r0:r0 + ROWS, :]),
            "v": np.ascontiguousarray(uv[0, 1, r0:r0 + ROWS, :]),
            "tbl": tbl,
        })

    res = bass_utils.run_bass_kernel_spmd(
        nc, in_maps, core_ids=list(range(N_CORES)))
    globals()["_LAST_RES"] = res
    parts = []
    for i in range(N_CORES):
        o = res.results[i]["out"].reshape(ROWS, FULL_W, C)
        parts.append(np.transpose(o, (2, 0, 1)).astype(np.float32))
    out = np.concatenate(parts, axis=1)[None]
    return out
